# revision 10
# baseline (speedup 1.0000x reference)
"""Multi-head attention (dense transformer block) for 8 Trainium2 NeuronCores.

Problem: x [4, 2048, 1024] f32, w_qkv [3072, 1024], w_out [1024, 1024]
  qkv = x @ w_qkv.T ; split q,k,v ; 16 heads x 64 dims
  out = softmax(q k^T / 8) v ; y = out @ w_out.T
Sharding: 8 shards = (batch b in 0..3) x (head-half hh in 0..1).
Each core handles one batch and 8 heads end-to-end: QKV projection
column-split, attention for its 8 heads, out-projection row-split ->
partial y. Host sums the two partial y's per batch. No collectives.

Schedule: the attention inner loop is ScalarE-gated (exp ACTIVATE
[128,1024] ~1.0us/k-tile vs 0.85us of PE score+AV work), so all
non-attention matmuls (next pair's q/k projection, out-projection of the
previous tq) are interleaved ONE MATMUL PER K-TILE SLOT as filler inside
the kt loop. The PE then always has ready work while exp latency is
absorbed. PSUM: scores 2banks x2, AV accum 1bank x2, qk-proj 1, outproj 1.

    v | qk(0) | pair0: attn + qk(1) filler | pair1: attn + qk(2)+outproj(0,1)
      | pair2: attn + qk(3)+outproj rest | pair3: attn + outproj(2,3) | tail

Out-projection lags attention by one tq (reads outT written by the
normalization epilogue); y accumulated in DRAM: pair 1 writes, pair 3
DMA-accumulates, chained per region.
"""

import numpy as np
from collections import deque

B = 4
NT = 2048          # tokens per batch
E = 1024           # embed dim
H = 16             # heads
DH = 64            # head dim
HD = 512           # head dims per core (8 heads)
N_CORES = 8
SCALE = DH ** -0.5
P = 128

_cache = {}


def _build(rep=1, ablate=(), mmdt="f32r", loop=False):
    import concourse.mybir as mybir
    import concourse.tile as tile
    from concourse import bacc
    from contextlib import ExitStack

    f32 = mybir.dt.float32
    _qk = {"f32r": mybir.dt.float32r, "bf16": mybir.dt.bfloat16,
           "fp16": mybir.dt.float16, "mix": mybir.dt.float16}
    _soft = {"f32r": mybir.dt.bfloat16, "bf16": mybir.dt.bfloat16,
             "fp16": mybir.dt.float16, "mix": mybir.dt.bfloat16}
    f32r = _qk[mmdt]          # q/k-side matmul dtype (x, wq, wk, wv, qT, kT)
    bf16 = _soft[mmdt]        # softmax/out-side dtype (es, vaug, outT, woT)
    in_dt = {"f32r": f32, "bf16": mybir.dt.bfloat16,
             "fp16": mybir.dt.float16, "mix": mybir.dt.float16}[mmdt]
    wo_dt = {"f32r": f32, "bf16": mybir.dt.bfloat16,
             "fp16": mybir.dt.float16, "mix": mybir.dt.bfloat16}[mmdt]
    Exp = mybir.ActivationFunctionType.Exp
    Add = mybir.AluOpType.add

    nc = bacc.Bacc("TRN2", target_bir_lowering=False, debug=False,
                   enable_asserts=False, num_devices=N_CORES)

    xT_ap = nc.dram_tensor("xT", [E, NT], in_dt, kind="ExternalInput").ap()
    wqT_ap = nc.dram_tensor("wqT", [E, HD], in_dt, kind="ExternalInput").ap()
    wkT_ap = nc.dram_tensor("wkT", [E, HD], in_dt, kind="ExternalInput").ap()
    wvT_ap = nc.dram_tensor("wvT", [E, HD], in_dt, kind="ExternalInput").ap()
    woT_ap = nc.dram_tensor("woT", [HD, E], wo_dt, kind="ExternalInput").ap()
    y_ap = nc.dram_tensor("y", [NT, E], f32, kind="ExternalOutput").ap()

    KE = E // P        # 8 contraction tiles over embed
    MQ = HD // P       # 4 partition tiles over head dims = head pairs
    TQ = NT // 512     # 4 query chunks of 512
    TT = NT // P       # 16 token tiles of 128

    from concourse.tile_rust import add_dep_helper

    with tile.TileContext(nc) as tc, ExitStack() as ctx:
        per = ctx.enter_context(tc.tile_pool(name="per", bufs=1))
        qk_pool = ctx.enter_context(tc.tile_pool(name="qk", bufs=4))
        outT_pool = ctx.enter_context(tc.tile_pool(name="ot", bufs=4))
        es_pool = ctx.enter_context(tc.tile_pool(name="es", bufs=4))
        y_pool = ctx.enter_context(tc.tile_pool(name="ysb", bufs=2))
        nrm_pool = ctx.enter_context(tc.tile_pool(name="nrm", bufs=2))
        bcs_pool = ctx.enter_context(tc.tile_pool(name="bcs", bufs=2))
        xT_pool = ctx.enter_context(tc.tile_pool(name="xTp", bufs=2))
        psS = ctx.enter_context(tc.tile_pool(name="psS", bufs=2, space="PSUM"))
        psAV = ctx.enter_context(tc.tile_pool(name="psAV", bufs=2, space="PSUM"))
        psM = ctx.enter_context(tc.tile_pool(name="psM", bufs=1, space="PSUM"))
        psO = ctx.enter_context(tc.tile_pool(name="psO", bufs=1, space="PSUM"))

        # rep-invariant weights (wv first: the value projection runs first)
        wv = per.tile([P, KE, HD], f32r, tag="wv")
        nc.scalar.dma_start(wv[:], wvT_ap.rearrange("(o p) m -> p o m", p=P).bitcast(f32r))
        wq = per.tile([P, KE, HD], f32r, tag="wq")
        nc.scalar.dma_start(wq[:], wqT_ap.rearrange("(o p) m -> p o m", p=P).bitcast(f32r))
        wk = per.tile([P, KE, HD], f32r, tag="wk")
        nc.scalar.dma_start(wk[:], wkT_ap.rearrange("(o p) m -> p o m", p=P).bitcast(f32r))
        woT = per.tile([P, MQ, E], bf16, tag="woT")
        nc.scalar.dma_start(woT[:], woT_ap.rearrange("(o p) e -> p o e", p=P).bitcast(bf16))
        # two vaug parities: rep r reads parity r%2 while its pair-3 slots
        # host the NEXT rep's value projection into parity (r+1)%2
        vaug_sets = []
        for par in range(2):
            vaug_g = [per.tile([P, 4, 8, DH + 1], bf16, tag=f"vaug{par}{g}",
                               name=f"vaug{par}{g}") for g in range(TT // 4)]
            vaug_sets.append([vaug_g[tt // 4][:, tt % 4] for tt in range(TT)])
            for g in range(TT // 4):
                nc.vector.memset(vaug_g[g][:, :, :, DH:DH + 1], 1.0)

        # Tile does not order DMAs by DRAM range: chain each y region's
        # write DMAs explicitly across reps.
        y_prev_dma = {}
        xT_src = xT_ap.rearrange("(o p) t -> p o t", p=P).bitcast(f32r)

        def ptile(pool, name):
            tag = "m" if pool is psM else "o"
            return pool.tile([P, 512], f32, tag=tag, name=name)

        def load_xT():
            xTs = []
            for ke in range(KE):
                xk = xT_pool.tile([P, NT], f32r, tag=f"xT{ke}", name=f"xT{ke}")
                nc.scalar.dma_start(xk[:], xT_src[:, ke, :])
                xTs.append(xk)
            return xTs

        def alloc_qk(mq):
            qT = qk_pool.tile([P, NT], f32r, tag="qTp", name=f"qT{mq}")
            kT = qk_pool.tile([P, NT], f32r, tag="kTp", name=f"kT{mq}")
            return qT, kT

        def v_steps(xTs, vaugs, pools):
            # value projection, one yield per matmul; ScalarE copy on the 8th
            for tt in range(TT):
                ps = ptile(pools[tt % len(pools)], "vps")
                for i in range(KE):
                    ke = (i + tt) % KE
                    nc.tensor.matmul(ps[:], xTs[ke][:, tt * P:(tt + 1) * P],
                                     wv[:, ke, :], start=(i == 0), stop=(i == KE - 1))
                    if i == KE - 1:
                        nc.scalar.copy(vaugs[tt][:, :, 0:DH],
                                       ps[:].rearrange("p (h d) -> p h d", h=8))
                    yield

        def qk_steps(mq, qT, kT, xTs, pools):
            # q/k projections for pair mq, one yield per matmul
            pi = 0
            for dst, w in ((kT, wk), (qT, wq)):
                for tq in range(TQ):
                    ps = ptile(pools[pi % len(pools)], "qkps")
                    pi += 1
                    for i in range(KE):
                        nc.tensor.matmul(ps[:], w[:, i, mq * P:(mq + 1) * P],
                                         xTs[i][:, tq * 512:(tq + 1) * 512],
                                         start=(i == 0), stop=(i == KE - 1))
                        if i == KE - 1:
                            nc.vector.tensor_copy(dst[:, tq * 512:(tq + 1) * 512], ps[:])
                        yield

        def op_steps(outTs, tq, pools):
            # out-projection of one tq's tokens: 8 chunks x 4 matmuls
            # (all four head-pairs accumulated in PSUM; y written once)
            opi = 0
            for tt in range(tq * 4, tq * 4 + 4):
                for ec in range(E // 512):
                    esl = slice(ec * 512, (ec + 1) * 512)
                    ps = ptile(pools[opi % len(pools)], "ops")
                    opi += 1
                    for p_ in range(MQ):
                        nc.tensor.matmul(ps[:], outTs[p_][:, tt * P:(tt + 1) * P],
                                         woT[:, p_, esl],
                                         start=(p_ == 0), stop=(p_ == MQ - 1))
                        if p_ == MQ - 1:
                            ysb = y_pool.tile([P, 512], f32, tag="ysb", name="ysb")
                            nc.vector.tensor_copy(ysb[:], ps[:])
                            dma = nc.sync.dma_start(
                                y_ap[tt * P:(tt + 1) * P, esl], ysb[:])
                            if (tt, ec) in y_prev_dma:
                                add_dep_helper(dma.ins, y_prev_dma[(tt, ec)].ins,
                                               reason="y write order across reps")
                            y_prev_dma[(tt, ec)] = dma
                        yield

        def pull(q):
            while q:
                try:
                    next(q[0])
                    return True
                except StopIteration:
                    q.popleft()
            return False

        # filler queues persist across reps (cross-rep software pipelining)
        qk_q = deque()   # next pair's q/k projections      (psM)
        op_q = deque()   # out-projection, lags one tq      (psO)
        qk0_q = deque()  # NEXT REP's pair-0 q/k projection (psO, pairs<3)
        v_q = deque()    # NEXT REP's value projection      (psM, pair 3)
        carry = {}       # tiles handed to the next rep

        def emit_attn_tq(pair, qT, kT, outT, tq, vaugs, extras):
            qsl = slice(tq * 512, (tq + 1) * 512)
            av0 = psAV.tile([DH + 1, 512], f32, tag="av", name="av0")
            av1 = psAV.tile([DH + 1, 512], f32, tag="av", name="av1")

            def emit_av(kt, es):
                nc.tensor.matmul(av0[:], vaugs[kt][:, 2 * pair, :], es[:, 0, :],
                                 start=(kt == 0), stop=(kt == TT - 1))
                nc.tensor.matmul(av1[:], vaugs[kt][:, 2 * pair + 1, :], es[:, 1, :],
                                 start=(kt == 0), stop=(kt == TT - 1))

            # AV lags scores/exp by TWO k-tiles; filler matmuls (1 qk-proj
            # slot guaranteed + up to 2 pulls from each extra queue) keep the
            # PE busy through the exp latency and the epilogue drain of the
            # previous tq's PSUM accumulators.
            pending = deque()
            for kt in range(TT):
                ksl = slice(kt * P, (kt + 1) * P)
                sps = psS.tile([P, 2, 512], f32, tag="s", name="sps")
                nc.tensor.matmul(sps[:, 0, :], kT[0:DH, ksl],
                                 qT[0:DH, qsl], start=True, stop=True)
                nc.tensor.matmul(sps[:, 1, :], kT[DH:P, ksl],
                                 qT[DH:P, qsl], start=True, stop=True)
                es = es_pool.tile([P, 2, 512], bf16, tag="es", name="es")
                nc.scalar.activation(es[:], sps[:], Exp, scale=SCALE)
                pull(qk_q)
                for q in extras:
                    pull(q)
                    pull(q)
                if len(pending) == 2:
                    emit_av(*pending.popleft())
                pending.append((kt, es))
            while pending:
                emit_av(*pending.popleft())
            for j, av in ((0, av0), (1, av1)):
                recip = nrm_pool.tile([1, 512], f32, tag="recip", name="recip")
                nc.vector.reciprocal(recip[:], av[DH:DH + 1, :])
                bcs = bcs_pool.tile([DH, 512], f32, tag="bcs", name="bcs")
                nc.gpsimd.partition_broadcast(bcs[:], recip[:])
                nc.vector.tensor_tensor(
                    outT[j * DH:(j + 1) * DH, qsl],
                    av[0:DH, :], bcs[:], mybir.AluOpType.mult)

        def emit_body(r, rep_total):
            first, last = r == 0, r == rep_total - 1
            parity = r % 2
            vaugs = vaug_sets[parity]

            if "xTs" in carry:
                xTs = carry.pop("xTs")
            else:
                xTs = load_xT()
            if first:
                for _ in v_steps(xTs, vaugs, (psM, psO)):
                    pass
            if "qk0" in carry:
                qT, kT = carry.pop("qk0")
                while qk0_q:          # safety: finish any unhosted steps
                    pull(qk0_q)
            else:
                qT, kT = alloc_qk(0)
                for _ in qk_steps(0, qT, kT, xTs, (psM, psO)):
                    pass

            outTs = []
            for pair in range(MQ):
                outT = outT_pool.tile([P, NT], bf16, tag="outT", name=f"outT{pair}")
                outTs.append(outT)
                if pair + 1 < MQ:
                    nqT, nkT = alloc_qk(pair + 1)
                    qk_q.append(qk_steps(pair + 1, nqT, nkT, xTs, (psM,)))
                else:
                    nqT = nkT = None
                if pair == 1 and not last:
                    carry["xTs"] = load_xT()
                if pair == 2 and not last:
                    q0, k0 = alloc_qk(0)
                    carry["qk0"] = (q0, k0)
                    qk0_q.append(qk_steps(0, q0, k0, carry["xTs"], (psO,)))
                if pair == 3 and not last:
                    v_q.append(v_steps(carry["xTs"], vaug_sets[1 - parity], (psM,)))
                extras = (op_q, qk0_q) if pair < MQ - 1 else (op_q, v_q)
                for tq in range(TQ):
                    emit_attn_tq(pair, qT, kT, outT, tq, vaugs, extras)
                    if pair == MQ - 1:
                        op_q.append(op_steps(outTs, tq,
                                             (psO,) if not last else (psO, psM)))
                qT, kT = nqT, nkT
            if last:
                while op_q or qk_q or v_q or qk0_q:
                    pull(op_q) or pull(qk_q) or pull(v_q) or pull(qk0_q)

        if loop:
            with tc.For_i(0, rep, 1):
                emit_body(0, 1)
        else:
            for r in range(rep):
                emit_body(r, rep)

    nc.compile()
    return nc


MMDT = "bf16"


def _get_nc(rep=1, ablate=(), mmdt=None):
    mmdt = mmdt or MMDT
    key = ("nc", rep, tuple(sorted(ablate)), mmdt)
    if key not in _cache:
        _cache[key] = _build(rep, ablate, mmdt)
    return _cache[key]


def make_in_maps(x, w_qkv, w_out, mmdt=None):
    import ml_dtypes
    mmdt = mmdt or MMDT
    dt = {"f32r": np.float32, "bf16": ml_dtypes.bfloat16,
          "fp16": np.float16, "mix": np.float16}[mmdt]
    wo_np = {"f32r": np.float32, "bf16": ml_dtypes.bfloat16,
             "fp16": np.float16, "mix": ml_dtypes.bfloat16}[mmdt]
    x = np.asarray(x, dtype=np.float32).astype(dt)
    w_qkv = np.asarray(w_qkv, dtype=np.float32).astype(dt)
    w_out = np.asarray(w_out, dtype=np.float32).astype(wo_np)
    in_maps = []
    for c in range(N_CORES):
        b, hh = divmod(c, 2)
        hsl = slice(hh * HD, (hh + 1) * HD)
        in_maps.append({
            "xT": np.ascontiguousarray(x[b].T),
            "wqT": np.ascontiguousarray(w_qkv[0 * E:1 * E][hsl].T),
            "wkT": np.ascontiguousarray(w_qkv[1 * E:2 * E][hsl].T),
            "wvT": np.ascontiguousarray(w_qkv[2 * E:3 * E][hsl].T),
            "woT": np.ascontiguousarray(w_out[:, hsl].T),
        })
    return in_maps


def combine_outputs(results):
    y = np.empty((B, NT, E), dtype=np.float32)
    for b in range(B):
        y[b] = results[2 * b]["y"] + results[2 * b + 1]["y"]
    return y


def kernel(x, w_qkv, w_out):
    from concourse.bass_utils import run_bass_kernel_spmd
    nc = _get_nc()
    in_maps = make_in_maps(x, w_qkv, w_out)
    res = run_bass_kernel_spmd(nc, in_maps, core_ids=list(range(N_CORES)))
    return combine_outputs(res.results)


# revision 12
# speedup vs baseline: 29.7262x; 29.7262x over previous
"""Multi-head attention (dense transformer block) for 8 Trainium2 NeuronCores.

Problem: x [4, 2048, 1024] f32, w_qkv [3072, 1024], w_out [1024, 1024]
  qkv = x @ w_qkv.T ; split q,k,v ; 16 heads x 64 dims
  out = softmax(q k^T / 8) v ; y = out @ w_out.T
Sharding: 8 shards = (batch b in 0..3) x (head-half hh in 0..1).
Each core handles one batch and 8 heads end-to-end: QKV projection
column-split, attention for its 8 heads, out-projection row-split ->
partial y. Host sums the two partial y's per batch. No collectives.

Schedule: the attention inner loop is ScalarE-gated (exp ACTIVATE
[128,1024] ~1.0us/k-tile vs 0.85us of PE score+AV work), so all
non-attention matmuls (next pair's q/k projection, out-projection of the
previous tq) are interleaved ONE MATMUL PER K-TILE SLOT as filler inside
the kt loop. The PE then always has ready work while exp latency is
absorbed. PSUM: scores 2banks x2, AV accum 1bank x2, qk-proj 1, outproj 1.

    v | qk(0) | pair0: attn + qk(1) filler | pair1: attn + qk(2)+outproj(0,1)
      | pair2: attn + qk(3)+outproj rest | pair3: attn + outproj(2,3) | tail

Out-projection lags attention by one tq (reads outT written by the
normalization epilogue); y accumulated in DRAM: pair 1 writes, pair 3
DMA-accumulates, chained per region.
"""

import numpy as np
from collections import deque

B = 4
NT = 2048          # tokens per batch
E = 1024           # embed dim
H = 16             # heads
DH = 64            # head dim
HD = 512           # head dims per core (8 heads)
N_CORES = 8
SCALE = DH ** -0.5
P = 128

_cache = {}


def _build(rep=1, ablate=(), mmdt="f32r", loop=False):
    import concourse.mybir as mybir
    import concourse.tile as tile
    from concourse import bacc
    from contextlib import ExitStack

    f32 = mybir.dt.float32
    _qk = {"f32r": mybir.dt.float32r, "bf16": mybir.dt.bfloat16,
           "fp16": mybir.dt.float16, "mix": mybir.dt.float16}
    _soft = {"f32r": mybir.dt.bfloat16, "bf16": mybir.dt.bfloat16,
             "fp16": mybir.dt.float16, "mix": mybir.dt.bfloat16}
    f32r = _qk[mmdt]          # q/k-side matmul dtype (x, wq, wk, wv, qT, kT)
    bf16 = _soft[mmdt]        # softmax/out-side dtype (es, vaug, outT, woT)
    in_dt = {"f32r": f32, "bf16": mybir.dt.bfloat16,
             "fp16": mybir.dt.float16, "mix": mybir.dt.float16}[mmdt]
    wo_dt = {"f32r": f32, "bf16": mybir.dt.bfloat16,
             "fp16": mybir.dt.float16, "mix": mybir.dt.bfloat16}[mmdt]
    Exp = mybir.ActivationFunctionType.Exp
    Add = mybir.AluOpType.add

    nc = bacc.Bacc("TRN2", target_bir_lowering=False, debug=False,
                   enable_asserts=False, num_devices=N_CORES)

    xT_ap = nc.dram_tensor("xT", [E, NT], in_dt, kind="ExternalInput").ap()
    wqT_ap = nc.dram_tensor("wqT", [E, HD], in_dt, kind="ExternalInput").ap()
    wkT_ap = nc.dram_tensor("wkT", [E, HD], in_dt, kind="ExternalInput").ap()
    wvT_ap = nc.dram_tensor("wvT", [E, HD], in_dt, kind="ExternalInput").ap()
    woT_ap = nc.dram_tensor("woT", [HD, E], wo_dt, kind="ExternalInput").ap()
    y_ap = nc.dram_tensor("y", [NT, E], f32, kind="ExternalOutput").ap()

    KE = E // P        # 8 contraction tiles over embed
    MQ = HD // P       # 4 partition tiles over head dims = head pairs
    TQ = NT // 512     # 4 query chunks of 512
    TT = NT // P       # 16 token tiles of 128

    from concourse.tile_rust import add_dep_helper

    with tile.TileContext(nc) as tc, ExitStack() as ctx:
        per = ctx.enter_context(tc.tile_pool(name="per", bufs=1))
        qk_pool = ctx.enter_context(tc.tile_pool(name="qk", bufs=4))
        outT_pool = ctx.enter_context(tc.tile_pool(name="ot", bufs=4))
        es_pool = ctx.enter_context(tc.tile_pool(name="es", bufs=4))
        y_pool = ctx.enter_context(tc.tile_pool(name="ysb", bufs=2))
        nrm_pool = ctx.enter_context(tc.tile_pool(name="nrm", bufs=2))
        bcs_pool = ctx.enter_context(tc.tile_pool(name="bcs", bufs=2))
        xT_pool = ctx.enter_context(tc.tile_pool(name="xTp", bufs=2))
        psS = ctx.enter_context(tc.tile_pool(name="psS", bufs=2, space="PSUM"))
        psAV = ctx.enter_context(tc.tile_pool(name="psAV", bufs=2, space="PSUM"))
        psM = ctx.enter_context(tc.tile_pool(name="psM", bufs=1, space="PSUM"))
        psO = ctx.enter_context(tc.tile_pool(name="psO", bufs=1, space="PSUM"))

        # rep-invariant weights (wv first: the value projection runs first)
        wv = per.tile([P, KE, HD], f32r, tag="wv")
        nc.scalar.dma_start(wv[:], wvT_ap.rearrange("(o p) m -> p o m", p=P).bitcast(f32r))
        wq = per.tile([P, KE, HD], f32r, tag="wq")
        nc.scalar.dma_start(wq[:], wqT_ap.rearrange("(o p) m -> p o m", p=P).bitcast(f32r))
        wk = per.tile([P, KE, HD], f32r, tag="wk")
        nc.scalar.dma_start(wk[:], wkT_ap.rearrange("(o p) m -> p o m", p=P).bitcast(f32r))
        woT = per.tile([P, MQ, E], bf16, tag="woT")
        nc.scalar.dma_start(woT[:], woT_ap.rearrange("(o p) e -> p o e", p=P).bitcast(bf16))
        # two vaug parities: rep r reads parity r%2 while its pair-3 slots
        # host the NEXT rep's value projection into parity (r+1)%2
        vaug_sets = []
        for par in range(2):
            vaug_g = [per.tile([P, 4, 8, DH + 1], bf16, tag=f"vaug{par}{g}",
                               name=f"vaug{par}{g}") for g in range(TT // 4)]
            vaug_sets.append([vaug_g[tt // 4][:, tt % 4] for tt in range(TT)])
            for g in range(TT // 4):
                nc.vector.memset(vaug_g[g][:, :, :, DH:DH + 1], 1.0)

        # Tile does not order DMAs by DRAM range: chain each y region's
        # write DMAs explicitly across reps.
        y_prev_dma = {}
        xT_src = xT_ap.rearrange("(o p) t -> p o t", p=P).bitcast(f32r)

        def ptile(pool, name):
            tag = "m" if pool is psM else "o"
            return pool.tile([P, 512], f32, tag=tag, name=name)

        def load_xT():
            xTs = []
            for ke in range(KE):
                xk = xT_pool.tile([P, NT], f32r, tag=f"xT{ke}", name=f"xT{ke}")
                nc.scalar.dma_start(xk[:], xT_src[:, ke, :])
                xTs.append(xk)
            return xTs

        def alloc_qk(mq):
            qT = qk_pool.tile([P, NT], f32r, tag="qTp", name=f"qT{mq}")
            kT = qk_pool.tile([P, NT], f32r, tag="kTp", name=f"kT{mq}")
            return qT, kT

        def v_steps(xTs, vaugs, pools):
            # value projection, one yield per matmul; ScalarE copy on the 8th
            for tt in range(TT):
                ps = ptile(pools[tt % len(pools)], "vps")
                for i in range(KE):
                    ke = (i + tt) % KE
                    nc.tensor.matmul(ps[:], xTs[ke][:, tt * P:(tt + 1) * P],
                                     wv[:, ke, :], start=(i == 0), stop=(i == KE - 1))
                    if i == KE - 1:
                        nc.scalar.copy(vaugs[tt][:, :, 0:DH],
                                       ps[:].rearrange("p (h d) -> p h d", h=8))
                    yield

        def qk_steps(mq, qT, kT, xTs, pools):
            # q/k projections for pair mq, one yield per matmul
            pi = 0
            for dst, w in ((kT, wk), (qT, wq)):
                for tq in range(TQ):
                    ps = ptile(pools[pi % len(pools)], "qkps")
                    pi += 1
                    for i in range(KE):
                        nc.tensor.matmul(ps[:], w[:, i, mq * P:(mq + 1) * P],
                                         xTs[i][:, tq * 512:(tq + 1) * 512],
                                         start=(i == 0), stop=(i == KE - 1))
                        if i == KE - 1:
                            nc.vector.tensor_copy(dst[:, tq * 512:(tq + 1) * 512], ps[:])
                        yield

        def op_steps(outTs, tq, pools):
            # out-projection of one tq's tokens: 8 chunks x 4 matmuls
            # (all four head-pairs accumulated in PSUM; y written once)
            opi = 0
            for tt in range(tq * 4, tq * 4 + 4):
                for ec in range(E // 512):
                    esl = slice(ec * 512, (ec + 1) * 512)
                    ps = ptile(pools[opi % len(pools)], "ops")
                    opi += 1
                    for p_ in range(MQ):
                        nc.tensor.matmul(ps[:], outTs[p_][:, tt * P:(tt + 1) * P],
                                         woT[:, p_, esl],
                                         start=(p_ == 0), stop=(p_ == MQ - 1))
                        if p_ == MQ - 1:
                            ysb = y_pool.tile([P, 512], f32, tag="ysb", name="ysb")
                            nc.vector.tensor_copy(ysb[:], ps[:])
                            dma = nc.sync.dma_start(
                                y_ap[tt * P:(tt + 1) * P, esl], ysb[:])
                            if (tt, ec) in y_prev_dma:
                                add_dep_helper(dma.ins, y_prev_dma[(tt, ec)].ins,
                                               reason="y write order across reps")
                            y_prev_dma[(tt, ec)] = dma
                        yield

        def pull(q):
            while q:
                try:
                    next(q[0])
                    return True
                except StopIteration:
                    q.popleft()
            return False

        # filler queues persist across reps (cross-rep software pipelining)
        qk_q = deque()   # next pair's q/k projections      (psM)
        op_q = deque()   # out-projection, lags one tq      (psO)
        qk0_q = deque()  # NEXT REP's pair-0 q/k projection (psO, pairs<3)
        v_q = deque()    # NEXT REP's value projection      (psM, pair 3)
        carry = {}       # tiles handed to the next rep

        def emit_attn_tq(pair, qT, kT, outT, tq, vaugs, extras):
            qsl = slice(tq * 512, (tq + 1) * 512)
            av0 = psAV.tile([DH + 1, 512], f32, tag="av", name="av0")
            av1 = psAV.tile([DH + 1, 512], f32, tag="av", name="av1")

            def emit_av(kt, es):
                nc.tensor.matmul(av0[:], vaugs[kt][:, 2 * pair, :], es[:, 0, :],
                                 start=(kt == 0), stop=(kt == TT - 1))
                nc.tensor.matmul(av1[:], vaugs[kt][:, 2 * pair + 1, :], es[:, 1, :],
                                 start=(kt == 0), stop=(kt == TT - 1))

            # AV lags scores/exp by TWO k-tiles; filler matmuls (1 qk-proj
            # slot guaranteed + up to 2 pulls from each extra queue) keep the
            # PE busy through the exp latency and the epilogue drain of the
            # previous tq's PSUM accumulators.
            pending = deque()
            for kt in range(TT):
                ksl = slice(kt * P, (kt + 1) * P)
                sps = psS.tile([P, 2, 512], f32, tag="s", name="sps")
                nc.tensor.matmul(sps[:, 0, :], kT[0:DH, ksl],
                                 qT[0:DH, qsl], start=True, stop=True)
                nc.tensor.matmul(sps[:, 1, :], kT[DH:P, ksl],
                                 qT[DH:P, qsl], start=True, stop=True)
                es = es_pool.tile([P, 2, 512], bf16, tag="es", name="es")
                nc.scalar.activation(es[:], sps[:], Exp, scale=SCALE)
                pull(qk_q)
                for q in extras:
                    pull(q)
                    pull(q)
                if len(pending) == 2:
                    emit_av(*pending.popleft())
                pending.append((kt, es))
            while pending:
                emit_av(*pending.popleft())
            for j, av in ((0, av0), (1, av1)):
                recip = nrm_pool.tile([1, 512], f32, tag="recip", name="recip")
                nc.vector.reciprocal(recip[:], av[DH:DH + 1, :])
                bcs = bcs_pool.tile([DH, 512], f32, tag="bcs", name="bcs")
                nc.gpsimd.partition_broadcast(bcs[:], recip[:])
                nc.vector.tensor_tensor(
                    outT[j * DH:(j + 1) * DH, qsl],
                    av[0:DH, :], bcs[:], mybir.AluOpType.mult)

        def emit_body(r, rep_total):
            first, last = r == 0, r == rep_total - 1
            parity = r % 2
            vaugs = vaug_sets[parity]

            if "xTs" in carry:
                xTs = carry.pop("xTs")
            else:
                xTs = load_xT()
            if first:
                for _ in v_steps(xTs, vaugs, (psM, psO)):
                    pass
            if "qk0" in carry:
                qT, kT = carry.pop("qk0")
                while qk0_q:          # safety: finish any unhosted steps
                    pull(qk0_q)
            else:
                qT, kT = alloc_qk(0)
                for _ in qk_steps(0, qT, kT, xTs, (psM, psO)):
                    pass

            outTs = []
            for pair in range(MQ):
                outT = outT_pool.tile([P, NT], bf16, tag="outT", name=f"outT{pair}")
                outTs.append(outT)
                if pair + 1 < MQ:
                    nqT, nkT = alloc_qk(pair + 1)
                    qk_q.append(qk_steps(pair + 1, nqT, nkT, xTs, (psM,)))
                else:
                    nqT = nkT = None
                if pair == 1 and not last:
                    carry["xTs"] = load_xT()
                if pair == 2 and not last:
                    q0, k0 = alloc_qk(0)
                    carry["qk0"] = (q0, k0)
                    qk0_q.append(qk_steps(0, q0, k0, carry["xTs"], (psO,)))
                if pair == 3 and not last:
                    v_q.append(v_steps(carry["xTs"], vaug_sets[1 - parity], (psM,)))
                extras = (op_q, qk0_q) if pair < MQ - 1 else (op_q, v_q)
                for tq in range(TQ):
                    emit_attn_tq(pair, qT, kT, outT, tq, vaugs, extras)
                    if pair == MQ - 1:
                        op_q.append(op_steps(outTs, tq,
                                             (psO,) if not last else (psO, psM)))
                qT, kT = nqT, nkT
            if last:
                while op_q or qk_q or v_q or qk0_q:
                    pull(op_q) or pull(qk_q) or pull(v_q) or pull(qk0_q)

        if loop:
            with tc.For_i(0, rep, 1):
                emit_body(0, 1)
        else:
            for r in range(rep):
                emit_body(r, rep)

    nc.compile()
    return nc


MMDT = "bf16"


def _get_nc(rep=1, ablate=(), mmdt=None):
    mmdt = mmdt or MMDT
    key = ("nc", rep, tuple(sorted(ablate)), mmdt)
    if key not in _cache:
        _cache[key] = _build(rep, ablate, mmdt)
    return _cache[key]


def make_in_maps(x, w_qkv, w_out, mmdt=None):
    import ml_dtypes
    mmdt = mmdt or MMDT
    dt = {"f32r": np.float32, "bf16": ml_dtypes.bfloat16,
          "fp16": np.float16, "mix": np.float16}[mmdt]
    wo_np = {"f32r": np.float32, "bf16": ml_dtypes.bfloat16,
             "fp16": np.float16, "mix": ml_dtypes.bfloat16}[mmdt]
    x = np.asarray(x, dtype=np.float32).astype(dt)
    w_qkv = np.asarray(w_qkv, dtype=np.float32).astype(dt)
    w_out = np.asarray(w_out, dtype=np.float32).astype(wo_np)
    in_maps = []
    for c in range(N_CORES):
        b, hh = divmod(c, 2)
        hsl = slice(hh * HD, (hh + 1) * HD)
        in_maps.append({
            "xT": np.ascontiguousarray(x[b].T),
            "wqT": np.ascontiguousarray(w_qkv[0 * E:1 * E][hsl].T),
            "wkT": np.ascontiguousarray(w_qkv[1 * E:2 * E][hsl].T),
            "wvT": np.ascontiguousarray(w_qkv[2 * E:3 * E][hsl].T),
            "woT": np.ascontiguousarray(w_out[:, hsl].T),
        })
    return in_maps


def combine_outputs(results):
    y = np.empty((B, NT, E), dtype=np.float32)
    for b in range(B):
        y[b] = results[2 * b]["y"] + results[2 * b + 1]["y"]
    return y


def kernel(x, w_qkv, w_out):
    from concourse.bass_utils import run_bass_kernel_spmd
    nc = _get_nc()
    in_maps = make_in_maps(x, w_qkv, w_out)
    res = run_bass_kernel_spmd(nc, in_maps, core_ids=list(range(N_CORES)))
    return combine_outputs(res.results)
